# revision 7
# baseline (speedup 1.0000x reference)
"""Trainium2 Bass kernel for nn_Cffn_68478958568093 (dense_mlp).

out = x @ U_w.T + V(z),  z = a0 + continued_fraction(a[..,1:]),
a = (sigmoid(x @ gate_w.T) * x) @ ladder_w.T

Distribution: data-parallel over the 8192 tokens across 8 NeuronCores
(1024 tokens/core), weights replicated.  All on-chip compute is done in
feature-major (transposed) layout; the host transposes per-core shards in
and the final output back.

Precision: the continued fraction has eps-guarded poles that amplify
errors in `a` by ~1e5, so the gate and ladder matmuls use an fp16 hi/lo
split (3 matmul passes: hi*hi + 2^-11*(hi*lo + lo*hi)) which measures at
fp32-grade accuracy (~5e-7) on the PE.  The U path is tolerance-loose and
uses a single fp16 pass; the tiny V contraction (K=3) runs in exact fp32
on the vector engine (per-partition scalar FMA against broadcast z rows),
accumulating straight onto the U psum results.  fp32/f32r single-pass
matmuls are not used for the big GEMMs: fp32 costs 4 cycles/row and f32r
is only TF32-accurate.

Measured: max abs err 0.011 vs the fp32 jax reference (absmax 37.9, rel
2.8e-4) — the reference's own fp32-vs-fp64 envelope is 0.008.  HW time
~530 us (CoreSim cost model: 482 us; PE floor for the required precision
is ~458 us).
"""

import numpy as np
from contextlib import ExitStack

import concourse.bass as bass
import concourse.bacc as bacc
import concourse.mybir as mybir
import concourse.tile as tile
from concourse.bass_utils import run_bass_kernel_spmd
from concourse.masks import make_identity

NCORES = 8
D = 2048
TOKENS = 4 * 2048
TPC = TOKENS // NCORES      # tokens per core = 1024
KT = D // 128               # 16 contraction chunks
NDT = D // 128              # 16 output-row tiles
NTT = TPC // 128            # 8 token tiles of 128
L = 3
DEPTH = 5
LK = L * (DEPTH + 1)        # 18
EPS = 0.01
SC = 2048.0                 # 2^11 hi/lo split scale
F16 = mybir.dt.float16
F32 = mybir.dt.float32
AOP = mybir.AluOpType


def _split16(a):
    """fp32 array -> (hi fp16, lo' fp16) with lo' = (a - hi) * 2048."""
    hi = a.astype(np.float16)
    lo = ((a - hi.astype(np.float32)) * SC).astype(np.float16)
    return hi, lo


def _build_program():
    nc = bacc.Bacc()

    def dp(name, shape, dt, out=False):
        return nc.declare_dram_parameter(name, list(shape), dt, isOutput=out)

    d_xhi = dp("xhi", [KT, 128, TPC], F16)
    d_xlo = dp("xlo", [KT, 128, TPC], F16)
    d_ghi = dp("ghi", [NDT, 128, KT, 128], F16)   # [dt][p][k][o]
    d_glo = dp("glo", [NDT, 128, KT, 128], F16)
    d_uhi = dp("uhi", [NDT, 128, KT, 128], F16)
    d_lwhi = dp("lwhi", [128, KT, LK], F16)
    d_lwlo = dp("lwlo", [128, KT, LK], F16)
    d_vw = dp("vw", [128, NDT, L], F32)           # V_w rows by partition
    d_out = dp("outT", [D, TPC], F32, out=True)

    with tile.TileContext(nc) as tc, ExitStack() as ctx:
        persist = ctx.enter_context(tc.tile_pool(name="persist", bufs=1))
        drp = ctx.enter_context(tc.tile_pool(name="drs", bufs=1, space="DRAM"))

        xhi, xlo = [], []
        for k in range(KT):
            t = persist.tile([128, TPC], F16, tag=f"xhi{k}")
            nc.sync.dma_start(out=t, in_=d_xhi[:, :, :][k])
            xhi.append(t)
            t = persist.tile([128, TPC], F16, tag=f"xlo{k}")
            nc.sync.dma_start(out=t, in_=d_xlo[:, :, :][k])
            xlo.append(t)

        lwhi = persist.tile([128, KT, LK], F16, tag="lwhi")
        nc.sync.dma_start(out=lwhi, in_=d_lwhi[:, :, :])
        lwlo = persist.tile([128, KT, LK], F16, tag="lwlo")
        nc.sync.dma_start(out=lwlo, in_=d_lwlo[:, :, :])
        vw = persist.tile([128, NDT, L], F32, tag="vw")
        nc.sync.dma_start(out=vw, in_=d_vw[:, :, :])

        ident = persist.tile([128, 128], F32, tag="ident")
        make_identity(nc, ident)

        ghi_g, glo_g = [], []       # gated_x hi/lo, by row tile
        for k in range(KT):
            ghi_g.append(persist.tile([128, TPC], F16, name=f"gghi{k}", tag=f"gghi{k}"))
            glo_g.append(persist.tile([128, TPC], F16, name=f"gglo{k}", tag=f"gglo{k}"))
        zT = persist.tile([L, TPC], F32, tag="zT")
        zbc = persist.tile([128, L, TPC], F32, tag="zbc")

        # ---------------- Phase A: gated_x = sigmoid(x @ gate_w.T) * x -----
        with tc.tile_pool(name="gw", bufs=2) as gwp, \
             tc.tile_pool(name="psA", bufs=2, space="PSUM") as psA, \
             tc.tile_pool(name="epi", bufs=2) as epi:
            for dt in range(NDT):
                gh = gwp.tile([128, KT, 128], F16, tag="gh")
                nc.scalar.dma_start(out=gh, in_=d_ghi[:, :, :, :][dt])
                gl = gwp.tile([128, KT, 128], F16, tag="gl")
                nc.scalar.dma_start(out=gl, in_=d_glo[:, :, :, :][dt])

                pm = [psA.tile([128, 512], F32, name=f"pm{dt}_{t}", tag=f"pm{t}") for t in range(2)]
                pc = [psA.tile([128, 512], F32, name=f"pc{dt}_{t}", tag=f"pc{t}") for t in range(2)]
                for k in range(KT):
                    first, last = k == 0, k == KT - 1
                    for t in range(2):
                        sl = slice(t * 512, (t + 1) * 512)
                        nc.tensor.matmul(pm[t], gh[:, k, :], xhi[k][:, sl],
                                         start=first, stop=last)
                        nc.tensor.matmul(pc[t], gh[:, k, :], xlo[k][:, sl],
                                         start=first, stop=False)
                    for t in range(2):
                        sl = slice(t * 512, (t + 1) * 512)
                        nc.tensor.matmul(pc[t], gl[:, k, :], xhi[k][:, sl],
                                         start=False, stop=last)

                g32 = epi.tile([128, TPC], F32, tag="g32")
                sig = epi.tile([128, TPC], F32, tag="sig")
                for t in range(2):
                    sl = slice(t * 512, (t + 1) * 512)
                    nc.vector.tensor_copy(g32[:, sl], pm[t])
                    nc.vector.scalar_tensor_tensor(
                        out=g32[:, sl], in0=pc[t], scalar=1.0 / SC,
                        in1=g32[:, sl], op0=AOP.mult, op1=AOP.add)
                    nc.scalar.activation(sig[:, sl], g32[:, sl],
                                         mybir.ActivationFunctionType.Sigmoid)
                x32 = epi.tile([128, TPC], F32, tag="x32")
                nc.vector.scalar_tensor_tensor(
                    out=x32, in0=xlo[dt], scalar=1.0 / SC, in1=xhi[dt],
                    op0=AOP.mult, op1=AOP.add)
                nc.vector.tensor_mul(g32, sig, x32)
                nc.vector.tensor_copy(ghi_g[dt], g32)
                d32 = epi.tile([128, TPC], F32, tag="d32")
                nc.vector.scalar_tensor_tensor(
                    out=d32, in0=ghi_g[dt], scalar=-1.0, in1=g32,
                    op0=AOP.mult, op1=AOP.add)
                nc.vector.tensor_scalar_mul(glo_g[dt], d32, SC)

        # ---------------- Phase B: a = gated @ lw.T ; CF ; z ---------------
        with tc.tile_pool(name="cfb", bufs=1) as cfb, \
             tc.tile_pool(name="psB", bufs=2, space="PSUM") as psB:
            a32 = cfb.tile([LK, TPC], F32, tag="a32")
            for t in range(2):
                sl = slice(t * 512, (t + 1) * 512)
                pam = psB.tile([LK, 512], F32, tag="pam")
                pac = psB.tile([LK, 512], F32, tag="pac")
                for k in range(KT):
                    first, last = k == 0, k == KT - 1
                    nc.tensor.matmul(pam, lwhi[:, k, :], ghi_g[k][:, sl],
                                     start=first, stop=last)
                    nc.tensor.matmul(pac, lwhi[:, k, :], glo_g[k][:, sl],
                                     start=first, stop=False)
                    nc.tensor.matmul(pac, lwlo[:, k, :], ghi_g[k][:, sl],
                                     start=False, stop=last)
                nc.vector.tensor_copy(a32[:, sl], pam)
                nc.vector.scalar_tensor_tensor(
                    out=a32[:, sl], in0=pac, scalar=1.0 / SC, in1=a32[:, sl],
                    op0=AOP.mult, op1=AOP.add)

            # transpose a to token-major [128, tt, l, k]
            at = cfb.tile([128, NTT, L, DEPTH + 1], F32, tag="at")
            for tt in range(NTT):
                pt = psB.tile([128, LK], F32, tag="pt")
                nc.tensor.transpose(
                    pt, a32[:, tt * 128:(tt + 1) * 128], ident[:LK, :LK])
                nc.vector.tensor_copy(
                    at[:, tt, :, :].rearrange("p l k -> p (l k)"), pt)

            # continued fraction with eps-guarded denominators
            f = cfb.tile([128, NTT, L], F32, tag="f")
            t1 = cfb.tile([128, NTT, L], F32, tag="t1")
            dc = cfb.tile([128, NTT, L], F32, tag="dc")
            msk = cfb.tile([128, NTT, L], mybir.dt.uint8, tag="msk")
            rc = cfb.tile([128, NTT, L], F32, tag="rc")
            nc.vector.tensor_copy(f, at[:, :, :, DEPTH])
            for kk in range(DEPTH - 1, 0, -1):
                nc.vector.tensor_scalar(out=t1, in0=f, scalar1=1.0,
                                        scalar2=EPS, op0=AOP.add, op1=AOP.max)
                nc.vector.tensor_scalar(out=dc, in0=f, scalar1=1.0,
                                        scalar2=-EPS, op0=AOP.add, op1=AOP.min)
                nc.vector.tensor_scalar(out=msk, in0=f, scalar1=1.0,
                                        scalar2=0.0, op0=AOP.add, op1=AOP.is_ge)
                nc.vector.copy_predicated(dc, msk, t1)
                nc.vector.reciprocal(rc, dc)
                nc.vector.tensor_mul(f, at[:, :, :, kk], rc)
            zt = cfb.tile([128, NTT, L], F32, tag="zt")
            nc.vector.tensor_add(zt, at[:, :, :, 0], f)

            for tt in range(NTT):
                pz = psB.tile([L, 128], F32, tag="pz")
                nc.tensor.transpose(pz, zt[:, tt, :], ident)
                nc.vector.tensor_copy(zT[:, tt * 128:(tt + 1) * 128], pz)
            z_dram = drp.tile([L, TPC], F32, tag="zdram")
            nc.sync.dma_start(out=z_dram, in_=zT)
            for l in range(L):
                nc.sync.dma_start(
                    out=zbc[:, l, :],
                    in_=z_dram[l:l + 1, :].to_broadcast([128, TPC]))

        # ---------------- Phase C: out = x @ U_w.T + z @ V_w.T -------------
        with tc.tile_pool(name="uw", bufs=2) as uwp, \
             tc.tile_pool(name="psC", bufs=2, space="PSUM") as psC, \
             tc.tile_pool(name="ob", bufs=3) as obp:
            for dt in range(NDT):
                ut = uwp.tile([128, KT, 128], F16, tag="ut")
                nc.scalar.dma_start(out=ut, in_=d_uhi[:, :, :, :][dt])
                po = [psC.tile([128, 512], F32, name=f"po{dt}_{t}", tag=f"po{t}") for t in range(2)]
                for k in range(KT):
                    for t in range(2):
                        sl = slice(t * 512, (t + 1) * 512)
                        nc.tensor.matmul(po[t], ut[:, k, :], xhi[k][:, sl],
                                         start=(k == 0), stop=(k == KT - 1))
                o32 = obp.tile([128, TPC], F32, tag="o32")
                for t in range(2):
                    sl = slice(t * 512, (t + 1) * 512)
                    nc.vector.scalar_tensor_tensor(
                        out=o32[:, sl], in0=zbc[:, 0, sl], scalar=vw[:, dt, 0:1],
                        in1=po[t], op0=AOP.mult, op1=AOP.add)
                    for l in range(1, L):
                        nc.vector.scalar_tensor_tensor(
                            out=o32[:, sl], in0=zbc[:, l, sl],
                            scalar=vw[:, dt, l:l + 1],
                            in1=o32[:, sl], op0=AOP.mult, op1=AOP.add)
                nc.sync.dma_start(
                    out=d_out[dt * 128:(dt + 1) * 128, :], in_=o32)

    nc.finalize()
    return nc


_NC_CACHE = {}


def _get_program():
    if "nc" not in _NC_CACHE:
        _NC_CACHE["nc"] = _build_program()
    return _NC_CACHE["nc"]


def make_in_maps(x, U_w, gate_w, ladder_w, V_w):
    """Host-side sharding + layout prep. Returns per-core input dicts."""
    x2 = np.ascontiguousarray(np.asarray(x, dtype=np.float32).reshape(TOKENS, D))

    def wtiles(w):
        # w: [out, in] fp32 -> fp16 tiles [dt][p][k][o] with
        # tile[dt, p, k, o] = w[dt*128+o, k*128+p]
        wT = w.T.astype(np.float32)                    # [d, o]
        a = wT.reshape(KT, 128, NDT, 128)              # [k, p, dt, o]
        return np.ascontiguousarray(a.transpose(2, 1, 0, 3))

    U_w = np.asarray(U_w, np.float32)
    gate_w = np.asarray(gate_w, np.float32)
    ladder_w = np.asarray(ladder_w, np.float32)
    V_w = np.asarray(V_w, np.float32)

    g_hi, g_lo = _split16(gate_w)
    ghi_t = wtiles(g_hi.astype(np.float32)).astype(np.float16)
    glo_t = wtiles(g_lo.astype(np.float32)).astype(np.float16)
    uhi_t = wtiles(U_w).astype(np.float16)

    lwT = ladder_w.transpose(2, 0, 1).reshape(D, LK)   # [d, (l k)]
    lw_hi, lw_lo = _split16(lwT)
    # [p, k, lk] with element (p,k,lk) = lwT[k*128+p, lk]
    lwhi_t = np.ascontiguousarray(
        lw_hi.reshape(KT, 128, LK).transpose(1, 0, 2))
    lwlo_t = np.ascontiguousarray(
        lw_lo.reshape(KT, 128, LK).transpose(1, 0, 2))

    vsc = np.ascontiguousarray(
        V_w.reshape(NDT, 128, L).transpose(1, 0, 2))   # [p, dt, l] fp32

    in_maps = []
    for c in range(NCORES):
        shard = x2[c * TPC:(c + 1) * TPC]              # [TPC, D]
        xT = np.ascontiguousarray(shard.T)             # [D, TPC]
        x_hi, x_lo = _split16(xT)
        in_maps.append({
            "xhi": np.ascontiguousarray(x_hi.reshape(KT, 128, TPC)),
            "xlo": np.ascontiguousarray(x_lo.reshape(KT, 128, TPC)),
            "ghi": ghi_t, "glo": glo_t, "uhi": uhi_t,
            "lwhi": lwhi_t, "lwlo": lwlo_t, "vw": vsc,
        })
    return in_maps


def assemble_output(results):
    parts = [results[c]["outT"].T for c in range(NCORES)]   # [TPC, D] each
    out = np.concatenate(parts, axis=0)                      # [TOKENS, D]
    return np.ascontiguousarray(out.reshape(4, 2048, D).astype(np.float32))


def kernel(x, U_w, gate_w, ladder_w, V_w):
    nc = _get_program()
    in_maps = make_in_maps(x, U_w, gate_w, ladder_w, V_w)
    res = run_bass_kernel_spmd(nc, in_maps, list(range(NCORES)))
    return assemble_output(res.results)


if __name__ == "__main__":
    rng = np.random.default_rng(0)
    x = rng.normal(0, 1, (4, 2048, D)).astype(np.float32)
    s = 1.0 / np.sqrt(D)
    U_w = rng.uniform(-s, s, (D, D)).astype(np.float32)
    gate_w = rng.uniform(-s, s, (D, D)).astype(np.float32)
    ladder_w = rng.uniform(-s, s, (L, DEPTH + 1, D)).astype(np.float32)
    V_w = rng.uniform(-1 / np.sqrt(L), 1 / np.sqrt(L), (D, L)).astype(np.float32)
    out = kernel(x=x, U_w=U_w, gate_w=gate_w, ladder_w=ladder_w, V_w=V_w)
    print("out", out.shape, out.dtype, np.abs(out).max())


# revision 9
# speedup vs baseline: 1.3624x; 1.3624x over previous
"""Trainium2 Bass kernel for nn_Cffn_68478958568093 (dense_mlp).

out = x @ U_w.T + V(z),  z = a0 + continued_fraction(a[..,1:]),
a = (sigmoid(x @ gate_w.T) * x) @ ladder_w.T

Distribution: data-parallel over the 8192 tokens across 8 NeuronCores
(1024 tokens/core), weights replicated.  All on-chip compute is done in
feature-major (transposed) layout; the host transposes per-core shards in
and the final output back.

Precision: the continued fraction has eps-guarded poles that amplify
errors in `a` by ~1e5, so the gate and ladder matmuls use an fp16 hi/lo
split (3 matmul passes: hi*hi + 2^-11*(hi*lo + lo*hi)) which measures at
fp32-grade accuracy (~5e-7) on the PE.  The U path is tolerance-loose and
uses a single fp16 pass; the tiny V contraction (K=3) runs in exact fp32
on the vector engine (per-partition scalar FMA against broadcast z rows),
accumulating straight onto the U psum results.  fp32/f32r single-pass
matmuls are not used for the big GEMMs: fp32 costs 4 cycles/row and f32r
is only TF32-accurate.

Measured: max abs err 0.011 vs the fp32 jax reference (absmax 37.9, rel
2.8e-4) — the reference's own fp32-vs-fp64 envelope is 0.008.  HW time
~530 us (CoreSim cost model: 482 us; PE floor for the required precision
is ~458 us).
"""

import numpy as np
from contextlib import ExitStack

import concourse.bass as bass
import concourse.bacc as bacc
import concourse.mybir as mybir
import concourse.tile as tile
from concourse.bass_utils import run_bass_kernel_spmd
from concourse.masks import make_identity

NCORES = 8
D = 2048
TOKENS = 4 * 2048
TPC = TOKENS // NCORES      # tokens per core = 1024
KT = D // 128               # 16 contraction chunks
NDT = D // 128              # 16 output-row tiles
NTT = TPC // 128            # 8 token tiles of 128
L = 3
DEPTH = 5
LK = L * (DEPTH + 1)        # 18
EPS = 0.01
SC = 2048.0                 # 2^11 hi/lo split scale
F16 = mybir.dt.float16
F32 = mybir.dt.float32
AOP = mybir.AluOpType


def _split16(a):
    """fp32 array -> (hi fp16, lo' fp16) with lo' = (a - hi) * 2048."""
    hi = a.astype(np.float16)
    lo = ((a - hi.astype(np.float32)) * SC).astype(np.float16)
    return hi, lo


def _build_program():
    nc = bacc.Bacc()

    def dp(name, shape, dt, out=False):
        return nc.declare_dram_parameter(name, list(shape), dt, isOutput=out)

    d_xhi = dp("xhi", [KT, 128, TPC], F16)
    d_xlo = dp("xlo", [KT, 128, TPC], F16)
    d_ghi = dp("ghi", [NDT, 128, KT, 128], F16)   # [dt][p][k][o]
    d_glo = dp("glo", [NDT, 128, KT, 128], F16)
    d_uhi = dp("uhi", [NDT, 128, KT, 128], F16)
    d_lwhi = dp("lwhi", [128, KT, LK], F16)
    d_lwlo = dp("lwlo", [128, KT, LK], F16)
    d_vw = dp("vw", [128, NDT, L], F32)           # V_w rows by partition
    d_out = dp("outT", [D, TPC], F32, out=True)

    with tile.TileContext(nc) as tc, ExitStack() as ctx:
        persist = ctx.enter_context(tc.tile_pool(name="persist", bufs=1))
        drp = ctx.enter_context(tc.tile_pool(name="drs", bufs=1, space="DRAM"))

        xhi, xlo = [], []
        for k in range(KT):
            t = persist.tile([128, TPC], F16, tag=f"xhi{k}")
            nc.sync.dma_start(out=t, in_=d_xhi[:, :, :][k])
            xhi.append(t)
            t = persist.tile([128, TPC], F16, tag=f"xlo{k}")
            nc.sync.dma_start(out=t, in_=d_xlo[:, :, :][k])
            xlo.append(t)

        lwhi = persist.tile([128, KT, LK], F16, tag="lwhi")
        nc.sync.dma_start(out=lwhi, in_=d_lwhi[:, :, :])
        lwlo = persist.tile([128, KT, LK], F16, tag="lwlo")
        nc.sync.dma_start(out=lwlo, in_=d_lwlo[:, :, :])
        vw = persist.tile([128, NDT, L], F32, tag="vw")
        nc.sync.dma_start(out=vw, in_=d_vw[:, :, :])

        ident = persist.tile([128, 128], F32, tag="ident")
        make_identity(nc, ident)

        ghi_g, glo_g = [], []       # gated_x hi/lo, by row tile
        for k in range(KT):
            ghi_g.append(persist.tile([128, TPC], F16, name=f"gghi{k}", tag=f"gghi{k}"))
            glo_g.append(persist.tile([128, TPC], F16, name=f"gglo{k}", tag=f"gglo{k}"))
        zT = persist.tile([L, TPC], F32, tag="zT")
        zbc = persist.tile([128, L, TPC], F32, tag="zbc")
        zt = persist.tile([128, NTT, L], F32, tag="zt")

        # ---------------- Phase A: gated_x = sigmoid(x @ gate_w.T) * x -----
        with tc.tile_pool(name="gw", bufs=2) as gwp, \
             tc.tile_pool(name="psA", bufs=2, space="PSUM") as psA, \
             tc.tile_pool(name="epi", bufs=2) as epi:
            for dt in range(NDT):
                gh = gwp.tile([128, KT, 128], F16, tag="gh")
                nc.scalar.dma_start(out=gh, in_=d_ghi[:, :, :, :][dt])
                gl = gwp.tile([128, KT, 128], F16, tag="gl")
                nc.scalar.dma_start(out=gl, in_=d_glo[:, :, :, :][dt])

                pm = [psA.tile([128, 512], F32, name=f"pm{dt}_{t}", tag=f"pm{t}") for t in range(2)]
                pc = [psA.tile([128, 512], F32, name=f"pc{dt}_{t}", tag=f"pc{t}") for t in range(2)]
                for k in range(KT):
                    first, last = k == 0, k == KT - 1
                    for t in range(2):
                        sl = slice(t * 512, (t + 1) * 512)
                        nc.tensor.matmul(pm[t], gh[:, k, :], xhi[k][:, sl],
                                         start=first, stop=last)
                        nc.tensor.matmul(pc[t], gh[:, k, :], xlo[k][:, sl],
                                         start=first, stop=False)
                    for t in range(2):
                        sl = slice(t * 512, (t + 1) * 512)
                        nc.tensor.matmul(pc[t], gl[:, k, :], xhi[k][:, sl],
                                         start=False, stop=last)

                g32 = epi.tile([128, TPC], F32, tag="g32")
                sig = epi.tile([128, TPC], F32, tag="sig")
                for t in range(2):
                    sl = slice(t * 512, (t + 1) * 512)
                    nc.vector.tensor_copy(g32[:, sl], pm[t])
                    nc.vector.scalar_tensor_tensor(
                        out=g32[:, sl], in0=pc[t], scalar=1.0 / SC,
                        in1=g32[:, sl], op0=AOP.mult, op1=AOP.add)
                    nc.scalar.activation(sig[:, sl], g32[:, sl],
                                         mybir.ActivationFunctionType.Sigmoid)
                x32 = epi.tile([128, TPC], F32, tag="x32")
                nc.vector.scalar_tensor_tensor(
                    out=x32, in0=xlo[dt], scalar=1.0 / SC, in1=xhi[dt],
                    op0=AOP.mult, op1=AOP.add)
                nc.vector.tensor_mul(g32, sig, x32)
                nc.vector.tensor_copy(ghi_g[dt], g32)
                d32 = epi.tile([128, TPC], F32, tag="d32")
                nc.vector.scalar_tensor_tensor(
                    out=d32, in0=ghi_g[dt], scalar=-1.0, in1=g32,
                    op0=AOP.mult, op1=AOP.add)
                nc.vector.tensor_scalar_mul(glo_g[dt], d32, SC)

        # ---------------- Phase B: a = gated @ lw.T ; CF ; z ---------------
        with tc.tile_pool(name="cfb", bufs=1) as cfb, \
             tc.tile_pool(name="psB", bufs=2, space="PSUM") as psB:
            a32 = cfb.tile([LK, TPC], F32, tag="a32")
            for t in range(2):
                sl = slice(t * 512, (t + 1) * 512)
                pam = psB.tile([LK, 512], F32, tag="pam")
                pac = psB.tile([LK, 512], F32, tag="pac")
                for k in range(KT):
                    first, last = k == 0, k == KT - 1
                    nc.tensor.matmul(pam, lwhi[:, k, :], ghi_g[k][:, sl],
                                     start=first, stop=last)
                    nc.tensor.matmul(pac, lwhi[:, k, :], glo_g[k][:, sl],
                                     start=first, stop=False)
                    nc.tensor.matmul(pac, lwlo[:, k, :], ghi_g[k][:, sl],
                                     start=False, stop=last)
                nc.vector.tensor_copy(a32[:, sl], pam)
                nc.vector.scalar_tensor_tensor(
                    out=a32[:, sl], in0=pac, scalar=1.0 / SC, in1=a32[:, sl],
                    op0=AOP.mult, op1=AOP.add)

            # transpose a to token-major [128, tt, l, k]
            at = cfb.tile([128, NTT, L, DEPTH + 1], F32, tag="at")
            for tt in range(NTT):
                pt = psB.tile([128, LK], F32, tag="pt")
                nc.tensor.transpose(
                    pt, a32[:, tt * 128:(tt + 1) * 128], ident[:LK, :LK])
                nc.vector.tensor_copy(
                    at[:, tt, :, :].rearrange("p l k -> p (l k)"), pt)

            # continued fraction with eps-guarded denominators
            f = cfb.tile([128, NTT, L], F32, tag="f")
            t1 = cfb.tile([128, NTT, L], F32, tag="t1")
            dc = cfb.tile([128, NTT, L], F32, tag="dc")
            msk = cfb.tile([128, NTT, L], mybir.dt.uint8, tag="msk")
            rc = cfb.tile([128, NTT, L], F32, tag="rc")
            nc.vector.tensor_copy(f, at[:, :, :, DEPTH])
            for kk in range(DEPTH - 1, 0, -1):
                nc.vector.tensor_scalar(out=t1, in0=f, scalar1=1.0,
                                        scalar2=EPS, op0=AOP.add, op1=AOP.max)
                nc.vector.tensor_scalar(out=dc, in0=f, scalar1=1.0,
                                        scalar2=-EPS, op0=AOP.add, op1=AOP.min)
                nc.vector.tensor_scalar(out=msk, in0=f, scalar1=1.0,
                                        scalar2=0.0, op0=AOP.add, op1=AOP.is_ge)
                nc.vector.copy_predicated(dc, msk, t1)
                nc.vector.reciprocal(rc, dc)
                nc.vector.tensor_mul(f, at[:, :, :, kk], rc)
            nc.vector.tensor_add(zt, at[:, :, :, 0], f)


        # ---------------- Phase C: out = x @ U_w.T + z @ V_w.T -------------
        # The z transposes + broadcast are emitted after the first two U
        # matmul groups (po triple-buffered), so the PE never idles while
        # the DVE finishes the continued fraction.
        with tc.tile_pool(name="uw", bufs=2) as uwp, \
             tc.tile_pool(name="psC", bufs=3, space="PSUM") as psC, \
             tc.tile_pool(name="ob", bufs=3) as obp:
            def emit_c_mms(dt):
                ut = uwp.tile([128, KT, 128], F16, name=f"ut{dt}", tag="ut")
                nc.scalar.dma_start(out=ut, in_=d_uhi[:, :, :, :][dt])
                po = [psC.tile([128, 512], F32, name=f"po{dt}_{t}", tag=f"po{t}") for t in range(2)]
                for k in range(KT):
                    for t in range(2):
                        sl = slice(t * 512, (t + 1) * 512)
                        nc.tensor.matmul(po[t], ut[:, k, :], xhi[k][:, sl],
                                         start=(k == 0), stop=(k == KT - 1))
                return po

            def emit_c_epilogue(dt, po):
                o32 = obp.tile([128, TPC], F32, name=f"o32_{dt}", tag="o32")
                for t in range(2):
                    sl = slice(t * 512, (t + 1) * 512)
                    nc.vector.scalar_tensor_tensor(
                        out=o32[:, sl], in0=zbc[:, 0, sl], scalar=vw[:, dt, 0:1],
                        in1=po[t], op0=AOP.mult, op1=AOP.add)
                    for l in range(1, L):
                        nc.vector.scalar_tensor_tensor(
                            out=o32[:, sl], in0=zbc[:, l, sl],
                            scalar=vw[:, dt, l:l + 1],
                            in1=o32[:, sl], op0=AOP.mult, op1=AOP.add)
                nc.sync.dma_start(
                    out=d_out[dt * 128:(dt + 1) * 128, :], in_=o32)

            po0 = emit_c_mms(0)
            po1 = emit_c_mms(1)
            for tt in range(NTT):
                pz = psC.tile([L, 128], F32, name=f"pz{tt}", tag="pz", bufs=2)
                nc.tensor.transpose(pz, zt[:, tt, :], ident)
                nc.vector.tensor_copy(zT[:, tt * 128:(tt + 1) * 128], pz)
            z_dram = drp.tile([L, TPC], F32, tag="zdram")
            nc.sync.dma_start(out=z_dram, in_=zT)
            for l in range(L):
                nc.sync.dma_start(
                    out=zbc[:, l, :],
                    in_=z_dram[l:l + 1, :].to_broadcast([128, TPC]))
            emit_c_epilogue(0, po0)
            emit_c_epilogue(1, po1)
            for dt in range(2, NDT):
                po = emit_c_mms(dt)
                emit_c_epilogue(dt, po)

    nc.finalize()
    return nc


_NC_CACHE = {}


def _get_program():
    if "nc" not in _NC_CACHE:
        _NC_CACHE["nc"] = _build_program()
    return _NC_CACHE["nc"]


def make_in_maps(x, U_w, gate_w, ladder_w, V_w):
    """Host-side sharding + layout prep. Returns per-core input dicts."""
    x2 = np.ascontiguousarray(np.asarray(x, dtype=np.float32).reshape(TOKENS, D))

    def wtiles(w):
        # w: [out, in] fp32 -> fp16 tiles [dt][p][k][o] with
        # tile[dt, p, k, o] = w[dt*128+o, k*128+p]
        wT = w.T.astype(np.float32)                    # [d, o]
        a = wT.reshape(KT, 128, NDT, 128)              # [k, p, dt, o]
        return np.ascontiguousarray(a.transpose(2, 1, 0, 3))

    U_w = np.asarray(U_w, np.float32)
    gate_w = np.asarray(gate_w, np.float32)
    ladder_w = np.asarray(ladder_w, np.float32)
    V_w = np.asarray(V_w, np.float32)

    g_hi, g_lo = _split16(gate_w)
    ghi_t = wtiles(g_hi.astype(np.float32)).astype(np.float16)
    glo_t = wtiles(g_lo.astype(np.float32)).astype(np.float16)
    uhi_t = wtiles(U_w).astype(np.float16)

    lwT = ladder_w.transpose(2, 0, 1).reshape(D, LK)   # [d, (l k)]
    lw_hi, lw_lo = _split16(lwT)
    # [p, k, lk] with element (p,k,lk) = lwT[k*128+p, lk]
    lwhi_t = np.ascontiguousarray(
        lw_hi.reshape(KT, 128, LK).transpose(1, 0, 2))
    lwlo_t = np.ascontiguousarray(
        lw_lo.reshape(KT, 128, LK).transpose(1, 0, 2))

    vsc = np.ascontiguousarray(
        V_w.reshape(NDT, 128, L).transpose(1, 0, 2))   # [p, dt, l] fp32

    in_maps = []
    for c in range(NCORES):
        shard = x2[c * TPC:(c + 1) * TPC]              # [TPC, D]
        xT = np.ascontiguousarray(shard.T)             # [D, TPC]
        x_hi, x_lo = _split16(xT)
        in_maps.append({
            "xhi": np.ascontiguousarray(x_hi.reshape(KT, 128, TPC)),
            "xlo": np.ascontiguousarray(x_lo.reshape(KT, 128, TPC)),
            "ghi": ghi_t, "glo": glo_t, "uhi": uhi_t,
            "lwhi": lwhi_t, "lwlo": lwlo_t, "vw": vsc,
        })
    return in_maps


def assemble_output(results):
    parts = [results[c]["outT"].T for c in range(NCORES)]   # [TPC, D] each
    out = np.concatenate(parts, axis=0)                      # [TOKENS, D]
    return np.ascontiguousarray(out.reshape(4, 2048, D).astype(np.float32))


def kernel(x, U_w, gate_w, ladder_w, V_w):
    nc = _get_program()
    in_maps = make_in_maps(x, U_w, gate_w, ladder_w, V_w)
    res = run_bass_kernel_spmd(nc, in_maps, list(range(NCORES)))
    return assemble_output(res.results)


if __name__ == "__main__":
    rng = np.random.default_rng(0)
    x = rng.normal(0, 1, (4, 2048, D)).astype(np.float32)
    s = 1.0 / np.sqrt(D)
    U_w = rng.uniform(-s, s, (D, D)).astype(np.float32)
    gate_w = rng.uniform(-s, s, (D, D)).astype(np.float32)
    ladder_w = rng.uniform(-s, s, (L, DEPTH + 1, D)).astype(np.float32)
    V_w = rng.uniform(-1 / np.sqrt(L), 1 / np.sqrt(L), (D, L)).astype(np.float32)
    out = kernel(x=x, U_w=U_w, gate_w=gate_w, ladder_w=ladder_w, V_w=V_w)
    print("out", out.shape, out.dtype, np.abs(out).max())
